# revision 1
# baseline (speedup 1.0000x reference)
"""Trainium2 Bass kernel for nn_DiffusionActionHead (MoE-style category routing).

Strategy (host side, inside kernel()):
  - Group the B=32 batch items by cat_id. Each distinct category's work is
    split into two column-halves (output-dim split of the big matmuls), giving
    uniform "half-unit" slots of ~6.4MB (bf16) weight traffic each.
  - Slots are distributed round-robin over the 8 NeuronCores; every core runs
    the SAME program over NSLOT slots (SPMD), with all routing baked into
    host-gathered per-core input arrays. Dummy padding slots replicate slot 0
    and their outputs are discarded.
  - Per-item sinusoidal timestep embeddings (a function of the int timesteps
    input only) are computed on host; all token/data-dependent FLOPs run on
    device. Host-side weight PREPROCESSING (input-independent, per category):
    the action-encoder first layer is algebraically folded into the second
    (a_emb enters layer 2 linearly through a rank-32 bottleneck):
        x2_pre = actions @ (ae_W1 @ ae_W2[:EMB, O]) + tau @ ae_W2[EMB:, O]
                 + (ae_b1 @ ae_W2[:EMB, O] + ae_b2[O])
    so the device loads a fused [32, 768] matrix per half instead of the
    1536x768 ae_W2[:EMB] half (-2.25MB/slot) and skips the AE1 phase.
  - All weights/activations are cast to bf16 on host: halves HBM traffic
    (the bottleneck; target_regime=memory). PSUM accumulation stays fp32;
    partial outputs return as bf16 and are summed on host in fp32 (total
    quantization noise ~5e-3 rel, gate 2e-2).
  - Column-half partial outputs are summed on host during unsharding.

Device program per slot (raw Bass, manual semaphores; bf16 matmuls):
  SE1  hT = relu(seW1h^T @ state + b1h)        (4x [128,4] matmuls)
  SE2  sf = hT^T @ seW2h + seb2(half0)         (partial state_feat, 3 o-tiles)
  TT   tt = tauT^T @ W2bh (+b2eff via bias mm) (per-item tau contribution)
  X2   x2T tiles [och,tok] = Wfused_chunk^T @ actionsT + ttT + b2eff;
       native Silu on ACT writes the bf16 AE3 stationary directly
       (6 [128,128] tiles rotating 4 PSUM banks; no transpose phase)
  AE3  out = x2T^T @ W3h + b3(half0)           (partial, 3 o-tiles of 512)

Weight streams: one 3.75MB DMA per slot carries wse2+w2b on the SP HWDGE
ring (plus 3 small pin DMAs); the w3 2.25MB prefetch and output DMAs ride
the ACT HWDGE ring. Elementwise copies/relu run on DVE so neither HWDGE
ring queues behind compute; big transfers keep HBM efficiency near peak.
"""
import sys

sys.path.insert(0, "/opt/trn_rl_repo")

import contextlib
import numpy as np
import ml_dtypes

import concourse.bass as bass
import concourse.mybir as mybir
from concourse.bass_utils import run_bass_kernel_spmd

F32 = mybir.dt.float32
BF16 = mybir.dt.bfloat16
NPBF16 = ml_dtypes.bfloat16
AF = mybir.ActivationFunctionType

E, STATE_DIM, ACT_DIM, HID, EMB = 32, 64, 32, 1024, 1536
B, T = 32, 32
N_CORES = 8
ITEMS_PER_SLOT = 4          # token tile = 4*32 = 128 tokens
HH = HID // 2               # 512: h-column half for the state encoder
OH = EMB // 2               # 768: output-column half for the action encoder
RS = 3                      # ring slots of [128, 15360] bf16: wse2+w2b (3 slots)
RB = 5                      # ring slots of [128, 9216] bf16: w3 (2.5 slots)

# pinA [64, 520]: se_W1 half chunks + stateT
PA_W1 = 0          # [0:64, 512]  4 chunks x 128
PA_ST = 512        # [0:64, 4]    stateT
PA_W = 520
# pinB [32, 896]: fused W1@W2a half + actionsT
PB_FU = 0          # [0:32, 768]
PB_ACT = 768       # [0:32, 128]  actionsT
PB_W = 896
# pinC [128, 64]: tauT chunks + se_b1 columns
PC_TAU = 0         # [128, 48]  12 k-chunks x 4 items
PC_SB1 = 48        # [128, 4]   se_b1 half, per-partition chunks
PC_W = 64

# BIAS row layout (free dim)
BIA_B2 = 0         # [768]  b2eff[O] = ae_b1 @ W2a[:, O] + ae_b2[O]
BIA_B3 = 768       # [1536] ae_b3 (half0 only)
BIA_SB2 = 2304     # [1536] se_b2 (half0 only)
BIA_W = 3840


def _sinusoid(ts):
    half = EMB // 2
    div = np.exp(-np.log(np.float32(10000.0)) * np.arange(half, dtype=np.float32) / np.float32(half))
    ang = ts.astype(np.float32)[:, None] * div[None, :]
    return np.concatenate([np.sin(ang), np.cos(ang)], axis=1).astype(np.float32)


# ---------------------------------------------------------------------------
# Build-time plan. Ops live in engine streams: "dma" (SP: input DMAs),
# "pe" (matmuls/transposes), "actq" (ACT: w3+output DMAs, relu/sigmoid),
# "dve" (copies/muls). Sem protocol: every DMA incs its own per-buffer sem by
# 16; every PE op incs s_pe by 1; every activation incs s_act by 1; every DVE
# op incs s_dve by 1. Cross-engine deps become standalone wait_ge ops.
# ---------------------------------------------------------------------------
class _Buf:
    __slots__ = ("writer", "readers")

    def __init__(self):
        self.writer = None      # (sem, value, stream)
        self.readers = []


class _Plan:
    def __init__(self):
        self.dma = []
        self.pe = []
        self.actq = []
        self.dve = []
        self.counts = {}

    def emit(self, stream, sem, mult, op, in_bufs, out_buf, force_wait=False):
        self.counts[sem] = self.counts.get(sem, 0) + 1
        tag = (sem, self.counts[sem] * mult, stream)
        deps = []
        for b in in_bufs:
            if b.writer is not None:
                deps.append(b.writer)
        if out_buf is not None:
            deps.extend(out_buf.readers)
            if out_buf.writer is not None:
                deps.append(out_buf.writer)
        m = {}
        for dsem, dval, dstream in deps:
            if dstream == stream and not force_wait:
                continue  # same engine stream: program order
            m[dsem] = max(m.get(dsem, 0), dval)
        op["waits"] = m
        getattr(self, stream).append(op)
        for b in in_bufs:
            b.readers.append(tag)
        if out_buf is not None:
            out_buf.writer = tag
            out_buf.readers = []


def build(nslot, reps=1, with_bias=False, with_b1=False, probe=None,
          swish_native=True):
    nc = bass.Bass()
    P = nc.declare_dram_parameter

    # Weights are stored host-side in chunk-major SBUF layout so each phase
    # needs one large contiguous DMA (18KB/partition rows for the big phases;
    # per-DMA fixed cost and descriptor tails amortize best on big transfers).
    # Per-buffer DMA semaphores keep narrow (sub-128-partition) transfers
    # sound: each buffer's "sem >= 16*n" fires exactly at its write n.
    pina = P("pina", [nslot, 64, PA_W], BF16, isOutput=False)
    pinb = P("pinb", [nslot, 32, PB_W], BF16, isOutput=False)
    pinc = P("pinc", [nslot, 128, PC_W], BF16, isOutput=False)
    # wse2 (4 chunks of 1536) and w2b (12 chunks of 768) ride ONE 3.75MB
    # transfer: cols 0:6144 = wse2, 6144:15360 = w2b
    wbig = P("wbig", [nslot, 128, 15360], BF16, isOutput=False)
    w3 = P("w3", [nslot, 128, 9216], BF16, isOutput=False)       # 6 chunks
    consts = P("consts", [128, 384], BF16, isOutput=False)       # iden|onesel|ones
    biasd = (P("biasd", [nslot, 128, BIA_W], BF16, isOutput=False)
             if with_bias else None)   # row 0 used
    ao = P("ao", [nslot, 128, EMB], BF16, isOutput=True)
    st = P("st", [nslot, ITEMS_PER_SLOT, EMB], BF16, isOutput=True)

    with contextlib.ExitStack() as es:
        ec = es.enter_context
        ring = [ec(nc.sbuf_tensor(f"ring{i}", [128, 15360], BF16)) for i in range(RS)]
        ringb = [ec(nc.sbuf_tensor(f"ringb{i}", [128, 9216], BF16)) for i in range(RB)]
        pa_b = [ec(nc.sbuf_tensor(f"pina{i}", [64, PA_W], BF16)) for i in range(2)]
        pb_b = [ec(nc.sbuf_tensor(f"pinb{i}", [32, PB_W], BF16)) for i in range(2)]
        pc_b = [ec(nc.sbuf_tensor(f"pinc{i}", [128, PC_W], BF16)) for i in range(2)]
        bias_b = ([ec(nc.sbuf_tensor(f"bias{i}", [128, BIA_W], BF16)) for i in range(2)]
                  if with_bias else [])
        cst_b = ec(nc.sbuf_tensor("cst_b", [128, 384], BF16))
        s_hT = ec(nc.sbuf_tensor("s_hT", [128, 16], BF16))
        s_sg = (None if swish_native else
                ec(nc.sbuf_tensor("s_sg", [128, OH], BF16)))
        s_tt = ec(nc.sbuf_tensor("s_tt", [ITEMS_PER_SLOT, OH], BF16))
        s_x2T = ec(nc.sbuf_tensor("s_x2T", [128, OH], BF16))
        s_out = [ec(nc.sbuf_tensor(f"s_out{i}", [128, EMB], BF16)) for i in range(2)]
        s_st = [ec(nc.sbuf_tensor(f"s_st{i}", [ITEMS_PER_SLOT, EMB], BF16)) for i in range(2)]
        pA = ec(nc.psum_tensor("pA", [128, 512], F32))
        pB0 = ec(nc.psum_tensor("pB0", [128, 512], F32))
        pB1 = ec(nc.psum_tensor("pB1", [128, 512], F32))
        pC = ec(nc.psum_tensor("pC", [128, 512], F32))
        pD = ec(nc.psum_tensor("pD", [128, 512], F32))
        pE = ec(nc.psum_tensor("pE", [128, 512], F32))
        pF = ec(nc.psum_tensor("pF", [128, 512], F32))
        pG = ec(nc.psum_tensor("pG", [128, 512], F32))
        s_pe = ec(nc.semaphore("s_pe"))
        s_act = ec(nc.semaphore("s_act"))
        s_dve = ec(nc.semaphore("s_dve"))
        block = ec(nc.Block())

        # ---------------- plan ----------------
        pl = _Plan()
        bufs = {
            "ring": [_Buf() for _ in range(RS)],
            "ringb": [_Buf() for _ in range(RB)],
            "pa": [_Buf() for _ in range(2)],
            "pb": [_Buf() for _ in range(2)],
            "pc": [_Buf() for _ in range(2)],
            "bias": [_Buf() for _ in range(2)],
            "hT": [_Buf() for _ in range(4)],
            "tt": [_Buf() for _ in range(2)],
            "sg": [_Buf() for _ in range(6)],
            "x2T": [_Buf() for _ in range(6)],
            "out": [_Buf() for _ in range(2)],
            "stb": [_Buf() for _ in range(2)],
            # Each psum tensor is one PSUM bank: PE writes and ACT/DVE reads
            # of the same bank are fatal if concurrent (P10), so track
            # whole-tensor — each new PE write waits for the prior reader.
            "pA": _Buf(),
            "pB0": _Buf(),
            "pB1": _Buf(),
            "pC": _Buf(),
            "pD": _Buf(),
            "pE": _Buf(),
            "pF": _Buf(),
            "pG": _Buf(),
            "consts": _Buf(),
        }
        rc = [0]
        rcb = [0]

        def next_ring():
            r = rc[0] % RS
            rc[0] += 1
            return r

        def next_ringb():
            r = rcb[0] % RB
            rcb[0] += 1
            return r

        def dma_in(dst, dst_sl, src, src_sl, buf, key, q="sp"):
            if q == "sp":
                pl.emit("dma", "dma:" + key, 16,
                        {"dst": dst, "dst_sl": dst_sl, "src": src, "src_sl": src_sl,
                         "key": "dma:" + key},
                        [], buf)
            else:
                pl.emit("actq", "dmo:" + key, 16,
                        {"kind": "dmo", "dst": dst, "dst_sl": dst_sl, "src": src,
                         "src_sl": src_sl, "key": "dmo:" + key},
                        [], buf)

        def dma_out(dst, dst_sl, src, src_sl, buf, key):
            pl.emit("actq", "dmo:" + key, 16,
                    {"kind": "dmo", "dst": dst, "dst_sl": dst_sl, "src": src,
                     "src_sl": src_sl, "key": "dmo:" + key}, [buf], None,
                    force_wait=True)

        def mm(out, out_sl, lhs, lhs_sl, rhs, rhs_sl, start, stop, in_bufs, out_buf):
            pl.emit("pe", "pe", 1,
                    {"kind": "mm", "out": out, "out_sl": out_sl, "lhs": lhs,
                     "lhs_sl": lhs_sl, "rhs": rhs, "rhs_sl": rhs_sl,
                     "start": start, "stop": stop}, in_bufs, out_buf)

        def tr(out, out_sl, in_, in_sl, in_bufs, out_buf):
            pl.emit("pe", "pe", 1,
                    {"kind": "tr", "out": out, "out_sl": out_sl, "in": in_,
                     "in_sl": in_sl}, in_bufs, out_buf)

        def act(out, out_sl, in_, in_sl, func, bias, in_bufs, out_buf):
            pl.emit("actq", "act", 1,
                    {"kind": "act", "out": out, "out_sl": out_sl, "in": in_,
                     "in_sl": in_sl, "func": func, "bias": bias}, in_bufs, out_buf)

        def dve(out, out_sl, in_, in_sl, in_bufs, out_buf, kind=None):
            op = {"out": out, "out_sl": out_sl, "in": in_, "in_sl": in_sl}
            if kind:
                op["kind"] = kind
            pl.emit("dve", "dve", 1, op, in_bufs, out_buf)

        # consts: one wide DMA. layout: [:,0:128]=iden, [0:4,128:256]=onesel,
        # [0:1,256:384]=ones row
        cb = bufs["consts"]
        dma_in("cst_b", np.s_[:, :], "consts", np.s_[:, :], cb, "cst")
        CS_IDEN, CS_SEL, CS_ONE = np.s_[:, 0:128], 128, 256

        def emit_slot(gs, s, emit_prev_out):
            # double-buffer index by GLOBAL slot count: with odd nslot, s % 2
            # would reuse the same buffer across the rep boundary, halving the
            # prefetch distance and stalling the SP queue ~12us every rep
            sb = gs % 2
            pab, pbb, pcb = bufs["pa"][sb], bufs["pb"][sb], bufs["pc"][sb]
            biab = bufs["bias"][sb]
            dma_in("pa_b", (sb, np.s_[:, :]), "pina", np.s_[s, :, :], pab, f"pa{sb}")
            dma_in("pb_b", (sb, np.s_[:, :]), "pinb", np.s_[s, :, :], pbb, f"pb{sb}")
            dma_in("pc_b", (sb, np.s_[:, :]), "pinc", np.s_[s, :, :], pcb, f"pc{sb}")
            if with_bias:
                dma_in("bias_b", (sb, np.s_[:, :]), "biasd", np.s_[s, :, :], biab, f"bias{sb}")

            # w3 (last-consumed phase) prefetches on the ACT HWDGE ring
            ra_3 = next_ringb()
            dma_in("ringb", (ra_3, np.s_[:, :]), "w3", np.s_[s, :, :], bufs["ringb"][ra_3], f"rb{ra_3}", q="act")

            # ---- SE1 ----
            for k in range(4):
                pn = ("pA", "pF")[k % 2]
                mm(pn, np.s_[0:128, k * 4:(k + 1) * 4],
                   "pa_b", (sb, np.s_[0:STATE_DIM, k * 128:(k + 1) * 128]),
                   "pa_b", (sb, np.s_[0:STATE_DIM, PA_ST:PA_ST + 4]),
                   True, True, [pab], bufs[pn])
                if with_b1:
                    act("s_hT", np.s_[:, k * 4:(k + 1) * 4], pn,
                        np.s_[0:128, k * 4:(k + 1) * 4],
                        AF.Relu, (sb, PC_SB1 + k), [bufs[pn], pcb], bufs["hT"][k])
                else:
                    # relu on DVE: keeps the slot spine independent of the ACT
                    # ring, whose w3 prefetch occupies it at slot start
                    dve("s_hT", np.s_[:, k * 4:(k + 1) * 4], pn,
                        np.s_[0:128, k * 4:(k + 1) * 4], [bufs[pn]], bufs["hT"][k],
                        kind="max")
            # ---- TT first (12 k-chunks from the wbig buffer): its 3.8us of
            # matmuls hide the SE1 relu round trip before SE2 needs s_hT, and
            # tt is ready long before X2's inject ----
            r1 = next_ring()
            dma_in("ring", (r1, np.s_[:, :]), "wbig", np.s_[s, :, :], bufs["ring"][r1], f"r{r1}")
            for k in range(12):
                for t, pn in enumerate(("pB0", "pB1")):
                    mm(pn, np.s_[0:ITEMS_PER_SLOT, 0:384],
                       "pc_b", (sb, np.s_[0:128, PC_TAU + k * 4:PC_TAU + (k + 1) * 4]),
                       "ring", (r1, np.s_[:, 6144 + k * 768 + t * 384:6144 + k * 768 + (t + 1) * 384]),
                       k == 0, (k == 11 and not with_bias),
                       [pcb, bufs["ring"][r1]], bufs[pn])
            if with_bias:
                for t, pn in enumerate(("pB0", "pB1")):
                    mm(pn, np.s_[0:ITEMS_PER_SLOT, 0:384],
                       "cst_b", np.s_[0:1, CS_ONE:CS_ONE + ITEMS_PER_SLOT],
                       "bias_b", (sb, np.s_[0:1, BIA_B2 + t * 384:BIA_B2 + (t + 1) * 384]),
                       False, True, [bufs["consts"], biab], bufs[pn])
            for t, pn in enumerate(("pB0", "pB1")):
                dve("s_tt", np.s_[0:ITEMS_PER_SLOT, t * 384:(t + 1) * 384],
                    pn, np.s_[0:ITEMS_PER_SLOT, 0:384], [bufs[pn]], bufs["tt"][t])
            # ---- SE2 (pC/pD/pG: pB0/pB1 stay with TT, then the X2 tile
            # rotation; pC/pD freed by the previous slot's out copies) ----
            for k in range(4):
                for t, pn in enumerate(("pC", "pD", "pG")):
                    mm(pn, np.s_[0:ITEMS_PER_SLOT, 0:512],
                       "s_hT", np.s_[:, k * 4:(k + 1) * 4],
                       "ring", (r1, np.s_[:, k * 1536 + t * 512:k * 1536 + (t + 1) * 512]),
                       k == 0, (k == 3 and not with_bias),
                       [bufs["hT"][k], bufs["ring"][r1]], bufs[pn])
            if with_bias:
                for t, pn in enumerate(("pC", "pD", "pG")):
                    mm(pn, np.s_[0:ITEMS_PER_SLOT, 0:512],
                       "cst_b", np.s_[0:1, CS_ONE:CS_ONE + ITEMS_PER_SLOT],
                       "bias_b", (sb, np.s_[0:1, BIA_SB2 + t * 512:BIA_SB2 + (t + 1) * 512]),
                       False, True, [bufs["consts"], biab], bufs[pn])
            for t, pn in enumerate(("pC", "pD", "pG")):
                dve("s_st", (sb, np.s_[0:ITEMS_PER_SLOT, t * 512:(t + 1) * 512]),
                    pn, np.s_[0:ITEMS_PER_SLOT, 0:512], [bufs[pn]], bufs["stb"][sb])

            # previous slot's output DMAs, mid-slot on the ACT ring
            emit_prev_out()

            # ---- X2 produced directly transposed: for each o-chunk t,
            # x2T tile [och 128, tok 128] = fused_chunk^T @ actionsT
            #                             + tt_chunk^T @ onesel (+ b2eff row).
            # Tiles ping-pong pA/pF (3 columns each) so the PE write of tile
            # t+2 overlaps ACT/DVE reads of tile t. No transpose phase needed.
            for t in range(6):
                pn = ("pA", "pF", "pB0", "pB1")[t % 4]
                qq = (t // 4) * 128
                tsl = np.s_[:, qq:qq + 128]
                mm(pn, tsl,
                   "pb_b", (sb, np.s_[0:ACT_DIM, PB_FU + t * 128:PB_FU + (t + 1) * 128]),
                   "pb_b", (sb, np.s_[0:ACT_DIM, PB_ACT:PB_ACT + 128]),
                   True, False, [pbb], bufs[pn])
                mm(pn, tsl,
                   "s_tt", np.s_[0:ITEMS_PER_SLOT, t * 128:(t + 1) * 128],
                   "cst_b", np.s_[0:ITEMS_PER_SLOT, CS_SEL:CS_SEL + 128],
                   False, not with_bias, [bufs["tt"][t // 3], bufs["consts"]], bufs[pn])
                if with_bias:
                    mm(pn, tsl,
                       "bias_b", (sb, np.s_[0:1, BIA_B2 + t * 128:BIA_B2 + (t + 1) * 128]),
                       "cst_b", np.s_[0:1, CS_ONE:CS_ONE + 128],
                       False, True, [biab, bufs["consts"]], bufs[pn])
                # swish via the ACT engine's native Silu table, straight into
                # the bf16 lhsT staging for AE3 (one engine hop, no DVE mul).
                # CoreSim lacks Silu: fall back to sigmoid + DVE mul there.
                if swish_native:
                    act("s_x2T", np.s_[:, t * 128:(t + 1) * 128], pn, tsl,
                        AF.Silu, None, [bufs[pn]], bufs["x2T"][t])
                else:
                    act("s_sg", np.s_[:, t * 128:(t + 1) * 128], pn, tsl,
                        AF.Sigmoid, None, [bufs[pn]], bufs["sg"][t])
                    pl.emit("dve", "dve", 1,
                            {"kind": "mul",
                             "out": "s_x2T", "out_sl": np.s_[:, t * 128:(t + 1) * 128],
                             "in": pn, "in_sl": tsl,
                             "in2": "s_sg", "in2_sl": np.s_[:, t * 128:(t + 1) * 128]},
                            [bufs[pn], bufs["sg"][t]], bufs["x2T"][t])
            # ---- AE3: w3 was prefetched on the ACT ring (ra_3) ----
            for k in range(6):
                for t, pn in enumerate(("pC", "pD", "pE")):
                    mm(pn, np.s_[:, 0:512], "s_x2T", np.s_[:, k * 128:(k + 1) * 128],
                       "ringb", (ra_3, np.s_[:, k * 1536 + t * 512:k * 1536 + (t + 1) * 512]),
                       k == 0, (k == 5 and not with_bias),
                       [bufs["x2T"][k], bufs["ringb"][ra_3]], bufs[pn])
            if with_bias:
                for t, pn in enumerate(("pC", "pD", "pE")):
                    mm(pn, np.s_[:, 0:512],
                       "cst_b", np.s_[0:1, CS_ONE:CS_ONE + 128],
                       "bias_b", (sb, np.s_[0:1, BIA_B3 + t * 512:BIA_B3 + (t + 1) * 512]),
                       False, True, [bufs["consts"], biab], bufs[pn])
            for t, pn in enumerate(("pC", "pD", "pE")):
                dve("s_out", (sb, np.s_[:, t * 512:(t + 1) * 512]), pn, np.s_[:, 0:512],
                    [bufs[pn]], bufs["out"][sb])

        def make_out_emitter(gs, s):
            def f():
                sb = gs % 2
                dma_out("ao", np.s_[s, :, :], "s_out", (sb, np.s_[:, :]), bufs["out"][sb], f"out{sb}")
                dma_out("st", np.s_[s, :, :], "s_st", (sb, np.s_[:, :]), bufs["stb"][sb], f"st{sb}")
            return f

        pending = lambda: None  # noqa: E731
        for rep in range(reps):
            for s in range(nslot):
                gs = rep * nslot + s
                emit_slot(gs, s, pending)
                pending = make_out_emitter(gs, s)
        pending()

        # ---------------- emit ----------------
        dma_sems = {k: ec(nc.semaphore("sem_" + k.replace(":", "_")))
                    for k in pl.counts if k.startswith(("dma:", "dmo:"))}

        tensors = {
            "ring": ring, "ringb": ringb, "pa_b": pa_b, "pb_b": pb_b, "pc_b": pc_b,
            "bias_b": bias_b, "cst_b": cst_b,
            "s_hT": s_hT, "s_tt": s_tt, "s_sg": s_sg, "s_x2T": s_x2T,
            "s_out": s_out, "s_st": s_st,
            "pA": pA, "pB0": pB0, "pB1": pB1, "pC": pC, "pD": pD, "pE": pE,
            "pF": pF, "pG": pG,
            "pina": pina, "pinb": pinb, "pinc": pinc, "wbig": wbig,
            "w3": w3, "biasd": biasd, "consts": consts,
            "ao": ao, "st": st,
        }

        def ap(name, sl):
            t = tensors[name]
            if isinstance(t, list):
                i, s2 = sl
                return t[i][s2]
            return t[sl]

        sems = {"pe": s_pe, "act": s_act, "dve": s_dve}

        def make_waiter(eng_handle):
            hw = {}

            def wait(wmap):
                for sname in sorted(wmap):
                    val = wmap[sname]
                    if hw.get(sname, 0) >= val:
                        continue
                    hw[sname] = val
                    h = sems[sname] if sname in sems else dma_sems[sname]
                    eng_handle.wait_ge(h, val)

            return wait

        if probe == "pe":
            pl.dma = []
        if probe in ("dma", "pe"):
            for _lst in (pl.dma, pl.pe, pl.actq, pl.dve):
                for _op in _lst:
                    _op["waits"] = {}
        if probe == "dma":
            # self-throttle: each DMA waits for the previous write to its own
            # buffer (ring depth flow control without compute)
            _kc = {}
            for _op in pl.dma:
                _k = _op["key"]
                if _kc.get(_k, 0) > 0:
                    _op["waits"] = {_k: 16 * _kc[_k]}
                _kc[_k] = _kc.get(_k, 0) + 1
        if probe == "dma":
            pl.pe = []
            pl.actq = [o for o in pl.actq if o["kind"] != "act"]
            pl.dve = [{"out": "s_hT", "out_sl": np.s_[0:4, 0:4],
                       "in": op["dst"],
                       "in_sl": (op["dst_sl"] if not isinstance(op["dst_sl"], tuple)
                                 or not isinstance(op["dst_sl"][0], int)
                                 else op["dst_sl"]),
                       "probe_read": True, "waits": {}}
                      for op in pl.dma]
            for op in pl.dve:
                sl = op["in_sl"]
                if isinstance(sl, tuple) and isinstance(sl[0], int):
                    op["in_sl"] = (sl[0], np.s_[0:4, 0:4])
                else:
                    op["in_sl"] = np.s_[0:4, 0:4]
        if probe == "pe":
            pl.actq = []
            pl.dve = []

        @block.sync
        def _(sync):
            wait = make_waiter(sync)
            cnt = {}
            for op in pl.dma:
                wait(op["waits"])
                k = op["key"]
                cnt[k] = cnt.get(k, 0) + 16
                sync.dma_start(out=ap(op["dst"], op["dst_sl"]),
                               in_=ap(op["src"], op["src_sl"])).then_inc(dma_sems[k], 16)
            for k, v in sorted(cnt.items()):
                sync.wait_ge(dma_sems[k], v)

        @block.tensor
        def _(pe):
            wait = make_waiter(pe)
            for op in pl.pe:
                wait(op["waits"])
                if op["kind"] == "mm":
                    pe.matmul(ap(op["out"], op["out_sl"]), ap(op["lhs"], op["lhs_sl"]),
                              ap(op["rhs"], op["rhs_sl"]), start=op["start"],
                              stop=op["stop"]).then_inc(s_pe, 1)
                else:
                    pe.transpose(ap(op["out"], op["out_sl"]), ap(op["in"], op["in_sl"]),
                                 cst_b[:, 0:128]).then_inc(s_pe, 1)

        @block.scalar
        def _(a):
            wait = make_waiter(a)
            dmo_cnt = {}
            for op in pl.actq:
                wait(op["waits"])
                if op["kind"] == "dmo":
                    k = op["key"]
                    dmo_cnt[k] = dmo_cnt.get(k, 0) + 16
                    a.dma_start(out=ap(op["dst"], op["dst_sl"]),
                                in_=ap(op["src"], op["src_sl"])).then_inc(dma_sems[k], 16)
                elif op["bias"] is None:
                    a.activation(ap(op["out"], op["out_sl"]), ap(op["in"], op["in_sl"]),
                                 op["func"]).then_inc(s_act, 1)
                else:
                    bi, bc = op["bias"]
                    bias_ap = pc_b[bi][:, bc:bc + 1]
                    a.activation(ap(op["out"], op["out_sl"]), ap(op["in"], op["in_sl"]),
                                 op["func"], bias=bias_ap).then_inc(s_act, 1)
            for k, v in sorted(dmo_cnt.items()):
                a.wait_ge(dma_sems[k], v)

        @block.vector
        def _(v):
            wait = make_waiter(v)
            for op in pl.dve:
                wait(op["waits"])
                if op.get("kind") == "mul":
                    v.tensor_mul(ap(op["out"], op["out_sl"]),
                                 ap(op["in"], op["in_sl"]),
                                 ap(op["in2"], op["in2_sl"])).then_inc(s_dve, 1)
                elif op.get("kind") == "max":
                    v.tensor_scalar_max(ap(op["out"], op["out_sl"]),
                                        ap(op["in"], op["in_sl"]),
                                        0.0).then_inc(s_dve, 1)
                else:
                    v.tensor_copy(ap(op["out"], op["out_sl"]),
                                  ap(op["in"], op["in_sl"])).then_inc(s_dve, 1)

    return nc


# ---------------------------------------------------------------------------
# Host-side routing, gathering, execution, unsharding
# ---------------------------------------------------------------------------
def plan_units(cat_ids):
    """Return list of units (cat, items(<=4), half) in a deterministic order."""
    order = {}
    for b, g in enumerate(cat_ids.tolist()):
        order.setdefault(g, []).append(b)
    units = []
    for g in sorted(order):
        items = order[g]
        for i0 in range(0, len(items), ITEMS_PER_SLOT):
            grp = items[i0:i0 + ITEMS_PER_SLOT]
            for h in range(2):
                units.append((g, grp, h))
    return units


def make_inputs(units_core, nslot, state, actions, tau_np,
                se_W1, se_b1, se_W2, se_b2,
                ae_W1, ae_b1, ae_W2, ae_b2, ae_W3, ae_b3, with_bias=None):
    if with_bias is None:
        with_bias = any(np.any(a) for a in (ae_b1, ae_b2, ae_b3, se_b2))
    z = np.zeros
    f = np.float32
    consts = z((128, 384), f)
    consts[:, 0:128] = np.eye(128, dtype=f)
    consts[0:ITEMS_PER_SLOT, 128:256] = np.kron(np.eye(ITEMS_PER_SLOT, dtype=f),
                                                np.ones((1, T), f))
    consts[0, 256:384] = 1.0
    d = {
        "pina": z((nslot, 64, PA_W), f),
        "pinb": z((nslot, 32, PB_W), f),
        "pinc": z((nslot, 128, PC_W), f),
        "wbig": z((nslot, 128, 15360), f),
        "w3": z((nslot, 128, 9216), f),
        "consts": consts,
    }

    def chunk_major(w, groups, chunks, width):
        # [groups*chunks*128, width] -> [groups, 128, chunks*width]
        return (w.reshape(groups, chunks, 128, width)
                .transpose(0, 2, 1, 3).reshape(groups, 128, chunks * width))
    if with_bias:
        d["biasd"] = z((nslot, 128, BIA_W), f)
    # input-independent weight preprocessing, once per (category, half)
    fused_cache = {}
    for s, (g, items, h) in enumerate(units_core):
        H = slice(h * HH, (h + 1) * HH)
        O = slice(h * OH, (h + 1) * OH)
        if (g, h) not in fused_cache:
            w2a_half = ae_W2[g][:EMB, O].astype(np.float64)
            fused_cache[(g, h)] = (
                (ae_W1[g].astype(np.float64) @ w2a_half).astype(f),
                (ae_b1[g].astype(np.float64) @ w2a_half).astype(f),
            )
        fused, b1w2a = fused_cache[(g, h)]
        d["pina"][s][:, PA_W1:PA_W1 + 512] = se_W1[g][:, H]
        d["pinb"][s][:, PB_FU:PB_FU + OH] = fused
        d["wbig"][s][:, 0:6144] = chunk_major(se_W2[g][H, :], 1, 4, EMB)[0]
        d["wbig"][s][:, 6144:15360] = chunk_major(ae_W2[g][EMB:, O], 1, 12, OH)[0]
        d["w3"][s] = chunk_major(ae_W3[g][O, :], 1, 6, EMB)[0]
        pc = d["pinc"][s]
        pc[:, PC_SB1:PC_SB1 + 4] = se_b1[g][H].reshape(4, 128).T
        tau3 = pc[:, PC_TAU:PC_TAU + 48].reshape(128, 12, ITEMS_PER_SLOT)
        for i, b in enumerate(items):
            tau3[:, :, i] = tau_np[b].reshape(12, 128).T
            d["pinb"][s][:, PB_ACT + i * T:PB_ACT + (i + 1) * T] = actions[b].T
            d["pina"][s][:, PA_ST + i] = state[b, 0]
        if with_bias:
            bb = d["biasd"][s][0]
            bb[BIA_B2:BIA_B2 + OH] = ae_b2[g][O] + b1w2a
            if h == 0:
                bb[BIA_B3:BIA_B3 + EMB] = ae_b3[g]
                bb[BIA_SB2:BIA_SB2 + EMB] = se_b2[g]
    return {k: v.astype(NPBF16) for k, v in d.items()}


def kernel(state, actions, timesteps, cat_ids,
           se_W1, se_b1, se_W2, se_b2,
           ae_W1, ae_b1, ae_W2, ae_b2, ae_W3, ae_b3):
    args = [np.asarray(a) for a in (state, actions, timesteps, cat_ids, se_W1, se_b1,
                                    se_W2, se_b2, ae_W1, ae_b1, ae_W2, ae_b2, ae_W3, ae_b3)]
    (state, actions, timesteps, cat_ids, se_W1, se_b1, se_W2, se_b2,
     ae_W1, ae_b1, ae_W2, ae_b2, ae_W3, ae_b3) = args
    tau_np = _sinusoid(timesteps)

    units = plan_units(cat_ids)
    nslot = max(1, -(-len(units) // N_CORES))
    per_core = [[] for _ in range(N_CORES)]
    for i, u in enumerate(units):
        per_core[i % N_CORES].append(u)
    for c in range(N_CORES):
        while len(per_core[c]) < nslot:
            per_core[c].append(None)  # dummy

    with_bias = bool(any(np.any(a) for a in (ae_b1, ae_b2, ae_b3, se_b2)))
    with_b1 = bool(np.any(se_b1))
    in_maps = []
    for c in range(N_CORES):
        units_c = [(u if u is not None else units[0]) for u in per_core[c]]
        in_maps.append(make_inputs(units_c, nslot, state, actions, tau_np,
                                   se_W1, se_b1, se_W2, se_b2,
                                   ae_W1, ae_b1, ae_W2, ae_b2, ae_W3, ae_b3,
                                   with_bias=with_bias))

    nc = build(nslot, with_bias=with_bias, with_b1=with_b1)
    res = run_bass_kernel_spmd(nc, in_maps, list(range(N_CORES)))

    out = np.zeros((B, T + 1, EMB), np.float32)
    for c in range(N_CORES):
        ao = res.results[c]["ao"].astype(np.float32)
        stx = res.results[c]["st"].astype(np.float32)
        for s, u in enumerate(per_core[c]):
            if u is None:
                continue
            g, items, h = u
            for i, b in enumerate(items):
                out[b, 0] += stx[s, i]
                out[b, 1:] += ao[s, i * T:(i + 1) * T]
    return out

